# revision 7
# baseline (speedup 1.0000x reference)
"""Additive (Bahdanau) attention scores on 8 TRN2 NeuronCores.

scores[b,q,k] = sum_h w[h]*tanh( (queries@U)[b,q,h] + (keys@T)[b,k,h] + bias[h] ) + w_bias

Shapes (hardcoded): B=4, LQ=LK=512, H=128, fp32.

Sharding: 8 shards = (batch b in 0..3) x (query half in 0..1).
Each core computes its [256 q, 512 k] score block, emitted k-major
([512, 256], scores^T); the host transposes and reassembles.

Per-core device pipeline (partitions = H = 128):
  1. PE:  qU^T = U^T @ queries^T   [128h, 256q]   (one matmul)
          kT^T = T^T @ keys^T      [128h, 512k]   (one matmul)
  2. DVE: kTb = kT^T + bias        (per-partition scalar add, psum->sbuf)
  3. Main loop over 16-query chunks:
       DVE: s[:, qi*512:(qi+1)*512] = kTb + qU^T[:, q]   (tensor_scalar add, 2x fp32)
       ACT: t = tanh(s)  on the whole [128, 8192] chunk (bf16 out)
       PE:  per (q, k-block): score_ps[kb][:, q] = t_block^T @ w  ([128,1] column)
  4. DVE: scores^T tiles = score_ps + w_bias  (psum->sbuf), DMA out.

The ScalarEngine tanh (16.8M elems/core @ 128 lanes * 1.2GHz) is the
roofline; everything else overlaps under it.
"""

import numpy as np

import concourse.bass as bass
import concourse.bacc as bacc
import concourse.mybir as mybir
from concourse.tile import TileContext

F32 = mybir.dt.float32
BF16 = mybir.dt.bfloat16

B, LQ_FULL, LK, H = 4, 512, 512, 128
N_CORES = 8
LQ = LQ_FULL // 2          # per-core query count (256)
QC = 16                    # queries per chunk
N_CHUNK = LQ // QC         # 16
KBLK = LK // 128           # 4 k-blocks of 128

_NC_CACHE = None


def _build_nc():
    nc = bacc.Bacc()

    # Two concatenated inputs so each projection matmul depends on exactly
    # one DMA queue (walrus allows only one sync-wait on a LDWEIGHTS):
    #   inA = qT(256) | U(128) | b(1) | w(1) | wb(1)   -> [128, 387]
    #   inB = kT(512) | T(128)                          -> [128, 640]
    inA_d = nc.declare_dram_parameter("inA", [H, LQ + H + 3], F32, isOutput=False)
    inB_d = nc.declare_dram_parameter("inB", [H, LK + H], F32, isOutput=False)
    out_d = nc.declare_dram_parameter("out", [LK, LQ], F32, isOutput=True)

    with TileContext(nc) as tc:
        with (
            tc.tile_pool(name="const", bufs=1) as cpool,
            tc.tile_pool(name="s", bufs=2) as spool,
            tc.tile_pool(name="t", bufs=2) as tpool,
            tc.tile_pool(name="o", bufs=2) as opool,
            tc.tile_pool(name="ps_proj", bufs=1, space="PSUM") as pj_pool,
            tc.tile_pool(name="ps_score", bufs=1, space="PSUM") as sc_pool,
        ):
            # ---- constants / inputs ----
            inA_sb = cpool.tile([H, LQ + H + 3], F32)
            nc.sync.dma_start(out=inA_sb, in_=inA_d[:, :])
            inB_sb = cpool.tile([H, LK + H], F32)
            nc.sync.dma_start(out=inB_sb, in_=inB_d[:, :])

            qT_sb = inA_sb[:, 0:LQ]
            U_sb = inA_sb[:, LQ:LQ + H]
            b_sb = inA_sb[:, LQ + H:LQ + H + 1]
            w_sb = inA_sb[:, LQ + H + 1:LQ + H + 2]
            wb_sb = inA_sb[:, LQ + H + 2:LQ + H + 3]
            kT_sb = inB_sb[:, 0:LK]
            T_sb = inB_sb[:, LK:LK + H]

            w_bf = cpool.tile([H, 1], BF16)
            nc.vector.tensor_copy(out=w_bf[:, :], in_=w_sb)

            # ---- projections ----
            qU_ps = pj_pool.tile([H, LQ], F32)
            nc.tensor.matmul(qU_ps[:, :], lhsT=U_sb, rhs=qT_sb,
                             start=True, stop=True)
            qU_sb = cpool.tile([H, LQ], F32)
            nc.vector.tensor_copy(out=qU_sb[:, :], in_=qU_ps[:, :])

            kTp_ps = pj_pool.tile([H, LK], F32)
            nc.tensor.matmul(kTp_ps[:, :], lhsT=T_sb, rhs=kT_sb,
                             start=True, stop=True)
            kTb_sb = cpool.tile([H, LK], F32)
            nc.vector.tensor_scalar_add(out=kTb_sb[:, :], in0=kTp_ps[:, :],
                                        scalar1=b_sb)

            # ---- score accumulators: scores^T columns land here ----
            score_ps = [sc_pool.tile([H, LQ], F32, name=f"score{kb}")
                        for kb in range(KBLK)]

            # ---- main loop ----
            for c in range(N_CHUNK):
                s_t = spool.tile([H, QC * LK], F32)
                for qi in range(QC):
                    q = c * QC + qi
                    nc.vector.tensor_scalar_add(
                        out=s_t[:, qi * LK:(qi + 1) * LK],
                        in0=kTb_sb[:, :],
                        scalar1=qU_sb[:, q:q + 1],
                    )
                t_t = tpool.tile([H, QC * LK], BF16)
                nc.scalar.activation(t_t[:, :], s_t[:, :],
                                     mybir.ActivationFunctionType.Tanh)
                for qi in range(QC):
                    q = c * QC + qi
                    for kb in range(KBLK):
                        nc.tensor.matmul(
                            score_ps[kb][:, q:q + 1],
                            lhsT=t_t[:, qi * LK + kb * 128: qi * LK + (kb + 1) * 128],
                            rhs=w_bf[:, :],
                            start=True, stop=True,
                        )

            # ---- evacuate + bias + store ----
            for kb in range(KBLK):
                o_sb = opool.tile([H, LQ], F32)
                nc.vector.tensor_scalar_add(out=o_sb[:, :], in0=score_ps[kb][:, :],
                                            scalar1=wb_sb)
                nc.sync.dma_start(out=out_d[kb * 128:(kb + 1) * 128, :], in_=o_sb[:, :])

    nc.compile()
    return nc


def get_nc():
    global _NC_CACHE
    if _NC_CACHE is None:
        _NC_CACHE = _build_nc()
    return _NC_CACHE


def make_in_maps(queries, keys, U, T, b, w, w_bias):
    queries = np.asarray(queries, np.float32)
    keys = np.asarray(keys, np.float32)
    U_c = np.asarray(U, np.float32)
    T_c = np.asarray(T, np.float32)
    b_c = np.asarray(b, np.float32).reshape(H, 1)
    w_c = np.asarray(w, np.float32).reshape(H, 1)
    wb_c = np.full((H, 1), np.float32(np.asarray(w_bias)), np.float32)

    in_maps = []
    for core in range(N_CORES):
        bb, qh = core // 2, core % 2
        qT = queries[bb, qh * LQ:(qh + 1) * LQ, :].T
        kT = keys[bb].T
        inA = np.ascontiguousarray(
            np.concatenate([qT, U_c, b_c, w_c, wb_c], axis=1))
        inB = np.ascontiguousarray(np.concatenate([kT, T_c], axis=1))
        in_maps.append({"inA": inA, "inB": inB})
    return in_maps


def assemble(results):
    out = np.empty((B, LQ_FULL, LK), np.float32)
    for core in range(N_CORES):
        bb, qh = core // 2, core % 2
        out[bb, qh * LQ:(qh + 1) * LQ, :] = results[core]["out"].T
    return out


def kernel(queries, keys, U, T, b, w, w_bias):
    from concourse.bass_utils import run_bass_kernel_spmd

    nc = get_nc()
    in_maps = make_in_maps(queries, keys, U, T, b, w, w_bias)
    res = run_bass_kernel_spmd(nc, in_maps, core_ids=list(range(N_CORES)))
    return assemble(res.results)


# revision 8
# speedup vs baseline: 1.0269x; 1.0269x over previous
"""Additive (Bahdanau) attention scores on 8 TRN2 NeuronCores.

scores[b,q,k] = sum_h w[h]*tanh( (queries@U)[b,q,h] + (keys@T)[b,k,h] + bias[h] ) + w_bias

Shapes (hardcoded): B=4, LQ=LK=512, H=128, fp32 in/out.

Sharding: 8 shards = (batch b in 0..3) x (query half in 0..1).
Each core computes its [256 q, 512 k] score block, emitted k-major
([512, 256], scores^T); the host transposes and reassembles.

Per-core device pipeline (partitions = H = 128):
  1. PE (bf16): qU^T = U^T @ queries^T [128h, 256q]; kT^T = T^T @ keys^T [128h, 512k]
  2. DVE: kTb = kT^T + bias (per-partition scalar add, psum->sbuf, bf16 out)
  3. Ramped chunk loop (q-chunk sizes 8,16,24,32...):
       DVE: s[:, qi*512:+512] = kTb + qU^T[:, q]  (bf16 tensor_scalar, 4x tier)
       ACT: t = tanh(s) over the whole chunk (the roofline: 1 elem/lane/cycle)
       PE:  per (q, k-block): score column [128,1] = t_block^T @ w  (FWL bf16)
       DVE: evacuate previous-parity chunk's score columns (+w_bias) -> DMA out
     PSUM score tiles are double-buffered (2 banks per parity) so PE writes
     and DVE evacuation never touch the same bank.

ScalarE tanh = 16.8M elems/core @ 128 lanes * 1.2GHz ~= 109us; everything
else (DVE s-build ~6.4us/32q, PE reduce ~35us total, DMA ~1MB) hides under it.
"""

import numpy as np
import ml_dtypes

import concourse.bass as bass
import concourse.bacc as bacc
import concourse.mybir as mybir
from concourse.tile import TileContext

F32 = mybir.dt.float32
BF16 = mybir.dt.bfloat16

B, LQ_FULL, LK, H = 4, 512, 512, 128
N_CORES = 8
LQ = LQ_FULL // 2          # per-core query count (256)

# Ramped chunk sizes: small head (first tanh starts early), small tail
# (last PE burst small). Sum must be LQ.
CHUNKS = [8, 16, 24] + [32] * 6 + [8, 8]
assert sum(CHUNKS) == LQ
QMAX = max(CHUNKS)

_NC_CACHE = None


def _build_nc():
    nc = bacc.Bacc()

    # inBF = qT(256) | U(128) | kT(512) | T(128)  bf16  -> [128, 1024]
    # inSC = b | w | wb                            f32   -> [128, 3]
    inBF_d = nc.declare_dram_parameter("inBF", [H, LQ + LK + 2 * H], BF16,
                                       isOutput=False)
    inSC_d = nc.declare_dram_parameter("inSC", [H, 3], F32, isOutput=False)
    out_d = nc.declare_dram_parameter("out", [LK, LQ], F32, isOutput=True)

    with TileContext(nc) as tc:
        with (
            tc.tile_pool(name="const", bufs=1) as cpool,
            tc.tile_pool(name="s", bufs=2) as spool,
            tc.tile_pool(name="t", bufs=2) as tpool,
            tc.tile_pool(name="o", bufs=8) as opool,
            tc.tile_pool(name="ps_proj", bufs=1, space="PSUM") as pj_pool,
            tc.tile_pool(name="ps_score", bufs=1, space="PSUM") as sc_pool,
        ):
            # ---- inputs ----
            inBF_sb = cpool.tile([H, LQ + LK + 2 * H], BF16)
            nc.sync.dma_start(out=inBF_sb, in_=inBF_d[:, :])
            inSC_sb = cpool.tile([H, 3], F32)
            nc.sync.dma_start(out=inSC_sb, in_=inSC_d[:, :])

            qT_bf = inBF_sb[:, 0:LQ]
            U_bf = inBF_sb[:, LQ:LQ + H]
            kT_bf = inBF_sb[:, LQ + H:LQ + H + LK]
            T_bf = inBF_sb[:, LQ + H + LK:LQ + H + LK + H]
            b_sb = inSC_sb[:, 0:1]
            w_sb = inSC_sb[:, 1:2]
            wb_sb = inSC_sb[:, 2:3]

            w_bf = cpool.tile([H, 1], BF16)
            nc.vector.tensor_copy(out=w_bf[:, :], in_=w_sb)

            # ---- projections (bf16 matmuls, fp32 psum) ----
            kTp_ps = pj_pool.tile([H, LK], F32)
            nc.tensor.matmul(kTp_ps[:, :], lhsT=T_bf, rhs=kT_bf,
                             start=True, stop=True)
            kTb_bf = cpool.tile([H, LK], BF16)
            nc.vector.tensor_scalar_add(out=kTb_bf[:, :], in0=kTp_ps[:, :],
                                        scalar1=b_sb)

            qU_ps = pj_pool.tile([H, LQ], F32)
            nc.tensor.matmul(qU_ps[:, :], lhsT=U_bf, rhs=qT_bf,
                             start=True, stop=True)
            qU_sb = cpool.tile([H, LQ], F32)  # fp32: feeds tensor_scalar scalar port
            nc.vector.tensor_copy(out=qU_sb[:, :], in_=qU_ps[:, :])

            # ---- PSUM score accumulators: 2 parities x 2 tiles x [128, 512] ----
            # tile j of a parity holds k-blocks 2j (cols 0:256) and 2j+1 (cols
            # 256:512); column index within a half = absolute q.
            score_ps = [[sc_pool.tile([H, 2 * LQ], F32, name=f"score{p}_{j}")
                         for j in range(2)] for p in range(2)]

            # ---- main loop over ramped chunks ----
            q0 = 0
            ranges = []
            for ci, qc in enumerate(CHUNKS):
                ranges.append((q0, qc))
                par = ci % 2
                s_t = spool.tile([H, QMAX * LK], BF16, name="s_t")
                for qi in range(qc):
                    q = q0 + qi
                    nc.vector.tensor_scalar_add(
                        out=s_t[:, qi * LK:(qi + 1) * LK],
                        in0=kTb_bf[:, :],
                        scalar1=qU_sb[:, q:q + 1],
                    )
                t_t = tpool.tile([H, QMAX * LK], BF16, name="t_t")
                nc.scalar.activation(t_t[:, 0:qc * LK], s_t[:, 0:qc * LK],
                                     mybir.ActivationFunctionType.Tanh)
                for qi in range(qc):
                    q = q0 + qi
                    for kb in range(4):
                        nc.tensor.matmul(
                            score_ps[par][kb // 2][:, (kb % 2) * LQ + q:
                                                   (kb % 2) * LQ + q + 1],
                            lhsT=t_t[:, qi * LK + kb * 128: qi * LK + (kb + 1) * 128],
                            rhs=w_bf[:, :],
                            start=True, stop=True,
                        )
                # evacuate the previous chunk of the SAME parity is not needed;
                # evacuate the PREVIOUS chunk (other parity) now that its PE
                # writes are long done.
                if ci >= 1:
                    eq0, eqc = ranges[ci - 1]
                    _evac(nc, opool, score_ps[(ci - 1) % 2], wb_sb, out_d,
                          eq0, eqc, ci - 1)
                q0 += qc
            # last chunk
            _evac(nc, opool, score_ps[(len(CHUNKS) - 1) % 2], wb_sb, out_d,
                  ranges[-1][0], ranges[-1][1], len(CHUNKS) - 1)

    nc.compile()
    return nc


def _evac(nc, opool, ps_tiles, wb_sb, out_d, q0, qc, ci):
    """Move score columns [q0:q0+qc] of one parity from PSUM to HBM, adding
    w_bias on the way."""
    for kb in range(4):
        o_sb = opool.tile([H, QMAX], F32, name=f"o_sb", tag="o_sb")
        nc.vector.tensor_scalar_add(
            out=o_sb[:, 0:qc],
            in0=ps_tiles[kb // 2][:, (kb % 2) * LQ + q0:(kb % 2) * LQ + q0 + qc],
            scalar1=wb_sb,
        )
        nc.sync.dma_start(out=out_d[kb * 128:(kb + 1) * 128, q0:q0 + qc],
                          in_=o_sb[:, 0:qc])


def get_nc():
    global _NC_CACHE
    if _NC_CACHE is None:
        _NC_CACHE = _build_nc()
    return _NC_CACHE


def make_in_maps(queries, keys, U, T, b, w, w_bias):
    queries = np.asarray(queries, np.float32)
    keys = np.asarray(keys, np.float32)
    U_c = np.asarray(U, np.float32)
    T_c = np.asarray(T, np.float32)
    b_c = np.asarray(b, np.float32).reshape(H, 1)
    w_c = np.asarray(w, np.float32).reshape(H, 1)
    wb_c = np.full((H, 1), np.float32(np.asarray(w_bias)), np.float32)
    inSC = np.ascontiguousarray(np.concatenate([b_c, w_c, wb_c], axis=1))

    in_maps = []
    for core in range(N_CORES):
        bb, qh = core // 2, core % 2
        qT = queries[bb, qh * LQ:(qh + 1) * LQ, :].T
        kT = keys[bb].T
        inBF = np.ascontiguousarray(
            np.concatenate([qT, U_c, kT, T_c], axis=1).astype(ml_dtypes.bfloat16))
        in_maps.append({"inBF": inBF, "inSC": inSC})
    return in_maps


def assemble(results):
    out = np.empty((B, LQ_FULL, LK), np.float32)
    for core in range(N_CORES):
        bb, qh = core // 2, core % 2
        out[bb, qh * LQ:(qh + 1) * LQ, :] = results[core]["out"].T
    return out


def kernel(queries, keys, U, T, b, w, w_bias):
    from concourse.bass_utils import run_bass_kernel_spmd

    nc = get_nc()
    in_maps = make_in_maps(queries, keys, U, T, b, w, w_bias)
    res = run_bass_kernel_spmd(nc, in_maps, core_ids=list(range(N_CORES)))
    return assemble(res.results)


# revision 9
# speedup vs baseline: 1.0676x; 1.0396x over previous
"""Additive (Bahdanau) attention scores on 8 TRN2 NeuronCores.

scores[b,q,k] = sum_h w[h]*tanh( (queries@U)[b,q,h] + (keys@T)[b,k,h] + bias[h] ) + w_bias

Shapes (hardcoded): B=4, LQ=LK=512, H=128, fp32 in/out.

Sharding: 8 shards = (batch b in 0..3) x (query half in 0..1).
Each core computes its [256 q, 512 k] score block, emitted k-major
([512, 256], scores^T); the host transposes and reassembles.

Per-core device pipeline (partitions = H = 128):
  1. PE (bf16): qU^T = U^T @ queries^T [128h, 256q]; kT^T = T^T @ keys^T [128h, 512k]
  2. DVE: kTb = kT^T + bias (per-partition scalar add, psum->sbuf, bf16 out)
  3. Ramped chunk loop (q-chunk sizes 8,16,24,32...):
       DVE: s[:, qi*512:+512] = kTb + qU^T[:, q]  (bf16 tensor_scalar, 4x tier)
       ACT: t = tanh(s) over the whole chunk (the roofline: 1 elem/lane/cycle)
       PE:  per (q, k-block): score column [128,1] = t_block^T @ w  (FWL bf16)
       DVE: evacuate previous-parity chunk's score columns (+w_bias) -> DMA out
     PSUM score tiles are double-buffered (2 banks per parity) so PE writes
     and DVE evacuation never touch the same bank.

ScalarE tanh = 16.8M elems/core @ 128 lanes * 1.2GHz ~= 109us; everything
else (DVE s-build ~6.4us/32q, PE reduce ~35us total, DMA ~1MB) hides under it.
"""

import numpy as np
import ml_dtypes

import concourse.bass as bass
import concourse.bacc as bacc
import concourse.mybir as mybir
from concourse.tile import TileContext

F32 = mybir.dt.float32
BF16 = mybir.dt.bfloat16

B, LQ_FULL, LK, H = 4, 512, 512, 128
N_CORES = 8
LQ = LQ_FULL // 2          # per-core query count (256)

# Ramped chunk sizes: small head (first tanh starts early), small tail
# (last PE burst small). Sum must be LQ.
CHUNKS = [8, 16, 24] + [32] * 5 + [24, 16, 8]
assert sum(CHUNKS) == LQ
QMAX = max(CHUNKS)

_NC_CACHE = None


def _build_nc():
    nc = bacc.Bacc()

    # inBF = kT(512) | T(128) | qT(256) | U(128)  bf16  -> [128, 1024]
    #   (k-side first: its DMA lands first and the kT projection starts early)
    # inSC = b | w | wb                             f32   -> [128, 3]
    inBF_d = nc.declare_dram_parameter("inBF", [H, LQ + LK + 2 * H], BF16,
                                       isOutput=False)
    inSC_d = nc.declare_dram_parameter("inSC", [H, 3], F32, isOutput=False)
    out_d = nc.declare_dram_parameter("out", [LK, LQ], F32, isOutput=True)

    with TileContext(nc) as tc:
        with (
            tc.tile_pool(name="const", bufs=1) as cpool,
            tc.tile_pool(name="s", bufs=2) as spool,
            tc.tile_pool(name="t", bufs=2) as tpool,
            tc.tile_pool(name="o", bufs=8) as opool,
            tc.tile_pool(name="ps_proj", bufs=1, space="PSUM") as pj_pool,
            tc.tile_pool(name="ps_score", bufs=1, space="PSUM") as sc_pool,
        ):
            # ---- inputs ----
            NBF = LQ + LK + 2 * H
            inBF_sb = cpool.tile([H, NBF], BF16)
            nc.sync.dma_start(out=inBF_sb[:, 0:LK + H], in_=inBF_d[:, 0:LK + H])
            nc.sync.dma_start(out=inBF_sb[:, LK + H:NBF], in_=inBF_d[:, LK + H:NBF])
            inSC_sb = cpool.tile([H, 3], F32)
            nc.sync.dma_start(out=inSC_sb, in_=inSC_d[:, :])

            kT_bf = inBF_sb[:, 0:LK]
            T_bf = inBF_sb[:, LK:LK + H]
            qT_bf = inBF_sb[:, LK + H:LK + H + LQ]
            U_bf = inBF_sb[:, LK + H + LQ:NBF]
            b_sb = inSC_sb[:, 0:1]
            w_sb = inSC_sb[:, 1:2]
            wb_sb = inSC_sb[:, 2:3]

            w_bf = cpool.tile([H, 1], BF16)
            nc.vector.tensor_copy(out=w_bf[:, :], in_=w_sb)

            # ---- projections (bf16 matmuls, fp32 psum) ----
            kTp_ps = pj_pool.tile([H, LK], F32)
            nc.tensor.matmul(kTp_ps[:, :], lhsT=T_bf, rhs=kT_bf,
                             start=True, stop=True)
            kTb_bf = cpool.tile([H, LK], BF16)
            nc.vector.tensor_scalar_add(out=kTb_bf[:, :], in0=kTp_ps[:, :],
                                        scalar1=b_sb)

            qU_ps = pj_pool.tile([H, LQ], F32)
            nc.tensor.matmul(qU_ps[:, :], lhsT=U_bf, rhs=qT_bf,
                             start=True, stop=True)
            qU_sb = cpool.tile([H, LQ], F32)  # fp32: feeds tensor_scalar scalar port
            nc.vector.tensor_copy(out=qU_sb[:, :], in_=qU_ps[:, :])

            # ---- PSUM score accumulators: 2 parities x 2 tiles x [128, 512] ----
            # tile j of a parity holds k-blocks 2j (cols 0:256) and 2j+1 (cols
            # 256:512); column index within a half = absolute q.
            score_ps = [[sc_pool.tile([H, 2 * LQ], F32, name=f"score{p}_{j}")
                         for j in range(2)] for p in range(2)]

            # ---- main loop over ramped chunks ----
            q0 = 0
            ranges = []
            for ci, qc in enumerate(CHUNKS):
                ranges.append((q0, qc))
                par = ci % 2
                s_t = spool.tile([H, QMAX * LK], BF16, name="s_t")
                for qi in range(qc):
                    q = q0 + qi
                    nc.vector.tensor_scalar_add(
                        out=s_t[:, qi * LK:(qi + 1) * LK],
                        in0=kTb_bf[:, :],
                        scalar1=qU_sb[:, q:q + 1],
                    )
                t_t = tpool.tile([H, QMAX * LK], BF16, name="t_t")
                nc.scalar.activation(t_t[:, 0:qc * LK], s_t[:, 0:qc * LK],
                                     mybir.ActivationFunctionType.Tanh)
                if ci >= 1:
                    eq0, eqc = ranges[ci - 1]
                    _evac(nc, opool, score_ps[(ci - 1) % 2], wb_sb, out_d,
                          eq0, eqc, ci - 1)
                for qi in range(qc):
                    q = q0 + qi
                    for kb in range(4):
                        nc.tensor.matmul(
                            score_ps[par][kb // 2][:, (kb % 2) * LQ + q:
                                                   (kb % 2) * LQ + q + 1],
                            lhsT=t_t[:, qi * LK + kb * 128: qi * LK + (kb + 1) * 128],
                            rhs=w_bf[:, :],
                            start=True, stop=True,
                        )
                q0 += qc
            # last chunk
            _evac(nc, opool, score_ps[(len(CHUNKS) - 1) % 2], wb_sb, out_d,
                  ranges[-1][0], ranges[-1][1], len(CHUNKS) - 1)

    nc.compile()
    return nc


def _evac(nc, opool, ps_tiles, wb_sb, out_d, q0, qc, ci):
    """Move score columns [q0:q0+qc] of one parity from PSUM to HBM, adding
    w_bias on the way."""
    for kb in range(4):
        o_sb = opool.tile([H, QMAX], F32, name=f"o_sb", tag="o_sb")
        nc.vector.tensor_scalar_add(
            out=o_sb[:, 0:qc],
            in0=ps_tiles[kb // 2][:, (kb % 2) * LQ + q0:(kb % 2) * LQ + q0 + qc],
            scalar1=wb_sb,
        )
        nc.sync.dma_start(out=out_d[kb * 128:(kb + 1) * 128, q0:q0 + qc],
                          in_=o_sb[:, 0:qc])


def get_nc():
    global _NC_CACHE
    if _NC_CACHE is None:
        _NC_CACHE = _build_nc()
    return _NC_CACHE


def make_in_maps(queries, keys, U, T, b, w, w_bias):
    queries = np.asarray(queries, np.float32)
    keys = np.asarray(keys, np.float32)
    U_c = np.asarray(U, np.float32)
    T_c = np.asarray(T, np.float32)
    b_c = np.asarray(b, np.float32).reshape(H, 1)
    w_c = np.asarray(w, np.float32).reshape(H, 1)
    wb_c = np.full((H, 1), np.float32(np.asarray(w_bias)), np.float32)
    inSC = np.ascontiguousarray(np.concatenate([b_c, w_c, wb_c], axis=1))

    in_maps = []
    for core in range(N_CORES):
        bb, qh = core // 2, core % 2
        qT = queries[bb, qh * LQ:(qh + 1) * LQ, :].T
        kT = keys[bb].T
        inBF = np.ascontiguousarray(
            np.concatenate([kT, T_c, qT, U_c], axis=1).astype(ml_dtypes.bfloat16))
        in_maps.append({"inBF": inBF, "inSC": inSC})
    return in_maps


def assemble(results):
    out = np.empty((B, LQ_FULL, LK), np.float32)
    for core in range(N_CORES):
        bb, qh = core // 2, core % 2
        out[bb, qh * LQ:(qh + 1) * LQ, :] = results[core]["out"].T
    return out


def kernel(queries, keys, U, T, b, w, w_bias):
    from concourse.bass_utils import run_bass_kernel_spmd

    nc = get_nc()
    in_maps = make_in_maps(queries, keys, U, T, b, w, w_bias)
    res = run_bass_kernel_spmd(nc, in_maps, core_ids=list(range(N_CORES)))
    return assemble(res.results)
